# revision 1
# baseline (speedup 1.0000x reference)
"""Trainium2 Bass kernel for nn_ContrastiveLoss (topk_masking, 8 cores).

Strategy (per sharding hint): shard the memory bank inputs_row/target_row
along M across the 8 cores (M_s = 8192 rows each). Each core computes its
[B, M_s] slice of the similarity matrix with the tensor engine, applies the
same-label masking fused into the PSUM->SBUF eviction (scalar_tensor_tensor),
accumulates the two relu-sums needed for the positive loss (ACT on PSUM +
GPSIMD on the masked matrix), and extracts per-512-chunk top-8 candidates
(DVE max8) which are merged into an exact per-shard top-16. The tiny
[B, 8*16] candidate set and per-shard partial sums are gathered to the host,
where the final re-topk (k=10) and mean are computed.

Layout trick: the host feeds inputs_row pre-transposed ([D, M_s], tiled to
[128, 4, M_s]) so both matmul operands already have the contraction dim D on
partitions - zero on-chip transposes.

pos-loss identity (per shard, per row b; c = 1 - eps):
  A  = sum_m relu(c - sim[b,m])                 (ACT accum, reads PSUM)
  Bm = sum_m min(negv[b,m] - c, 0) = -[ sum_{diff} relu(c - sim) + n_same*c ]
  pos = sum_{same} relu(c - sim) = A + Bm + n_same * c
n_same comes from a host-side label bincount (labels only, no sim needed).
"""

import numpy as np

B = 256
D = 512
M = 65536
NCORES = 8
MS = M // NCORES  # 8192
P = 128
KT = D // P  # 4 contraction tiles
MT = 512  # m-supertile (= top-k chunk size)
NMT = MS // MT  # 16
NBT = B // P  # 2
NEG_TOPK = 10
EPS = 1e-5
CTHR = 1.0 - EPS

_CACHE = {}


def _build_bass(reps=1):
    import concourse.bacc as bacc
    import concourse.mybir as mybir
    from concourse.tile import TileContext

    f32 = mybir.dt.float32
    Alu = mybir.AluOpType
    Act = mybir.ActivationFunctionType

    nc = bacc.Bacc("TRN2")
    colT = nc.dram_tensor("colT", [P, KT, B], f32, kind="ExternalInput")
    rowT = nc.dram_tensor("rowT", [P, KT, MS], f32, kind="ExternalInput")
    tcol = nc.dram_tensor("tcol", [P, NBT], f32, kind="ExternalInput")
    trow = nc.dram_tensor("trow", [1, MS], f32, kind="ExternalInput")
    cand_o = nc.dram_tensor("cand", [B, 16], f32, kind="ExternalOutput")
    sums_o = nc.dram_tensor("sums", [B, 2], f32, kind="ExternalOutput")

    with TileContext(nc) as tc:
        with (
            tc.tile_pool(name="const", bufs=1) as const,
            tc.tile_pool(name="rhs", bufs=4) as rhsp,
            tc.tile_pool(name="psum", bufs=6, space="PSUM") as psump,
            tc.tile_pool(name="neg", bufs=1) as negp,
            tc.tile_pool(name="small", bufs=3) as smallp,
        ):
            lhsT = const.tile([P, KT, B], f32)
            nc.sync.dma_start(lhsT[:], colT[:])
            tcS = const.tile([P, NBT], f32)
            nc.sync.dma_start(tcS[:], tcol[:])
            trR = const.tile([1, MS], f32)
            nc.sync.dma_start(trR[:], trow[:])
            trB = const.tile([P, MS], f32)
            # chunked so each broadcast overlaps the pipeline instead of
            # serializing ~17us of Pool work before the first eviction
            for mt in range(NMT):
                sl = slice(mt * MT, (mt + 1) * MT)
                nc.gpsimd.partition_broadcast(trB[:, sl], trR[:, sl])
            cthr = const.tile([P, 1], f32)
            nc.vector.memset(cthr[:], CTHR)

            for _rep in range(reps):
              negv = negp.tile([P, NBT, MS], f32, tag="negv")
              aacc = const.tile([P, NBT, NMT], f32, tag="aacc")
              bacc_t = const.tile([P, NBT, NMT], f32, tag="bacc")
              candt = const.tile([P, NBT, NMT, 8], f32, tag="candt")

              for mt in range(NMT):
                rhs = rhsp.tile([P, KT, MT], f32)
                nc.sync.dma_start(rhs[:], rowT[:, :, mt * MT : (mt + 1) * MT])
                for bt in range(NBT):
                    ps = psump.tile([P, MT], f32)
                    for kt in range(KT):
                        nc.tensor.matmul(
                            ps[:],
                            lhsT[:, kt, bt * P : (bt + 1) * P],
                            rhs[:, kt],
                            start=(kt == 0),
                            stop=(kt == KT - 1),
                        )
                    seg = negv[:, bt, mt * MT : (mt + 1) * MT]
                    # masked eviction: negv = (trow != tcol) * sim
                    nc.vector.scalar_tensor_tensor(
                        out=seg,
                        in0=trB[:, mt * MT : (mt + 1) * MT],
                        scalar=tcS[:, bt : bt + 1],
                        in1=ps[:],
                        op0=Alu.not_equal,
                        op1=Alu.mult,
                    )
                    # A accum: sum relu(c - sim), reading PSUM on ACT
                    u = smallp.tile([P, MT], f32, tag="u")
                    nc.scalar.activation(
                        u[:],
                        ps[:],
                        Act.Relu,
                        bias=cthr[:],
                        scale=-1.0,
                        accum_out=aacc[:, bt, mt : mt + 1],
                    )
                    # S_min accum: sum_m min(negv, c) on DVE (2x 1-input mode)
                    # (tensor_scalar: out = in0 op0 s1; accum = reduce_{op1}(out))
                    v = smallp.tile([P, MT], f32, tag="v")
                    nc.vector.tensor_scalar(
                        out=v[:],
                        in0=seg,
                        scalar1=CTHR,
                        scalar2=None,
                        op0=Alu.min,
                        op1=Alu.add,
                        accum_out=bacc_t[:, bt, mt : mt + 1],
                    )
                    # per-chunk top-8 candidates
                    nc.vector.max(out=candt[:, bt, mt], in_=seg)

            for bt in range(NBT):
                sb = smallp.tile([P, 2], f32, tag="sb")
                nc.vector.reduce_sum(
                    out=sb[:, 0:1], in_=aacc[:, bt], axis=mybir.AxisListType.X
                )
                nc.vector.reduce_sum(
                    out=sb[:, 1:2], in_=bacc_t[:, bt], axis=mybir.AxisListType.X
                )
                nc.sync.dma_start(sums_o[bt * P : (bt + 1) * P, :], sb[:])

                t8a = smallp.tile([P, 8], f32, tag="t8a")
                nc.vector.max(out=t8a[:], in_=candt[:, bt])
                c2 = smallp.tile([P, NMT, 8], f32, tag="c2")
                nc.vector.match_replace(
                    out=c2[:],
                    in_to_replace=t8a[:],
                    in_values=candt[:, bt],
                    imm_value=-1e30,
                )
                t8b = smallp.tile([P, 8], f32, tag="t8b")
                nc.vector.max(out=t8b[:], in_=c2[:])
                o16 = smallp.tile([P, 16], f32, tag="o16")
                nc.vector.tensor_copy(o16[:, 0:8], t8a[:])
                nc.vector.tensor_copy(o16[:, 8:16], t8b[:])
                nc.sync.dma_start(cand_o[bt * P : (bt + 1) * P, :], o16[:])

    nc.compile()
    return nc


def _get_bass():
    if "nc" not in _CACHE:
        _CACHE["nc"] = _build_bass()
    return _CACHE["nc"]


def _shard_inputs(inputs_col, targets_col, inputs_row, target_row):
    colT = (
        inputs_col.astype(np.float32)
        .T.reshape(KT, P, B)
        .transpose(1, 0, 2)
    )
    colT = np.ascontiguousarray(colT)
    tcol = np.ascontiguousarray(
        targets_col.astype(np.float32).reshape(NBT, P).T
    )
    in_maps = []
    for c in range(NCORES):
        sh = slice(c * MS, (c + 1) * MS)
        rowT = (
            inputs_row[sh]
            .astype(np.float32)
            .T.reshape(KT, P, MS)
            .transpose(1, 0, 2)
        )
        in_maps.append(
            {
                "colT": colT,
                "rowT": np.ascontiguousarray(rowT),
                "tcol": tcol,
                "trow": np.ascontiguousarray(
                    target_row[sh].astype(np.float32).reshape(1, MS)
                ),
            }
        )
    return in_maps


def _combine(results, targets_col, target_row):
    cands = np.concatenate([r["cand"] for r in results], axis=1)  # [B, 16*8]
    sums = np.stack([r["sums"] for r in results])  # [8, B, 2]
    counts = np.bincount(target_row.astype(np.int64), minlength=1)
    n_same = counts[np.minimum(targets_col.astype(np.int64), len(counts) - 1)]
    n_same = np.where(targets_col.astype(np.int64) < len(counts), n_same, 0)
    A = sums[:, :, 0].sum(axis=0, dtype=np.float64)
    Sm = sums[:, :, 1].sum(axis=0, dtype=np.float64)
    # pos = sum_same relu(c - sim); per shard: A_s + Smin_s - (MS - n_same_s)*c
    pos = A + Sm - (M - n_same.astype(np.float64)) * CTHR
    neg = np.sort(cands, axis=1)[:, -NEG_TOPK:].sum(axis=1, dtype=np.float64)
    return np.float32(np.mean(pos + neg))


def kernel(inputs_col, targets_col, inputs_row, target_row):
    from concourse.bass_utils import run_bass_kernel_spmd

    nc = _get_bass()
    in_maps = _shard_inputs(inputs_col, targets_col, inputs_row, target_row)
    res = run_bass_kernel_spmd(nc, in_maps, core_ids=list(range(NCORES)))
    return _combine(res.results, targets_col, target_row)



# revision 2
# speedup vs baseline: 3185.1399x; 3185.1399x over previous
"""Trainium2 Bass kernel for nn_ContrastiveLoss (topk_masking, 8 cores) — v3.

Strategy (per sharding hint): shard the memory bank inputs_row/target_row
along M across the 8 cores (M_s = 8192 rows each). Each core computes its
[B, M_s] slice of the similarity matrix in bf16 on the tensor engine,
processing PSUM in [128, 2048] rounds (4 banks, double-buffered). The
engine assignment keeps every elementwise pass in a DVE fast mode:

  ACT : evicts PSUM -> bf16 `simb` (its only job; PSUM reads are 1x-capped
        on DVE, so the f32->bf16 eviction goes to the otherwise-idle ACT)
  DVE : negv = (trow != tcol) * simb        (all-bf16 stt, 2x mode)
        accum S_all  = sum min(simb, c)     (tensor_scalar, 4x mode)
        fold = pairwise max of negv halves  (tensor_tensor, 2x mode)
        max8 over the folded 1024           (1x, but half the elements)
  Pool: accum S_diff = sum min(negv, c)     (gpsimd tensor_scalar)

Labels are remapped on the host (LUT) into distinct bf16-exact values so
the not_equal mask is exact in bf16. Both pos-loss sums read the same bf16
values, so bf16 rounding cancels exactly in the pos identity:

  pos = sum_same relu(c - simb) = n_same*c + (S_diff - S_all)

The pairwise-max fold before max8 can only lose a top-10 candidate when
two near-top values collide at the same fold index (P ~ 1e-4 per row, and
the replacement candidate differs by ~1 part in 1e3 of the row loss — the
validated rel-err stays ~1e-4).

The tiny [B, 4*8] candidates and [B, 8] partial sums go to the host, which
merges the 8 shards: bincount for n_same, top-10 of 256 candidates, mean.

Layouts: both matmul operands are fed pre-transposed from the host in bf16
(contraction dim D on partitions) — no on-chip transposes. rowB is grouped
[P, chunk, KT, 2048] so each 2MB chunk DMA moves 16KB-contiguous
per-partition lines. Matmuls run kt-outer so each loaded weight tile is
reused for 4 consecutive matmuls.
"""

import numpy as np
import ml_dtypes

BF16 = ml_dtypes.bfloat16

B = 256
D = 512
M = 65536
NCORES = 8
MS = M // NCORES  # 8192
P = 128
KT = D // P  # 4 contraction tiles
CHW = 2048  # eviction-round width (4 PSUM banks)
NCH = MS // CHW  # 4 chunks per shard
NSUB = CHW // 512  # 4 matmul sub-tiles per round
NBT = B // P  # 2
NEG_TOPK = 10
EPS = 1e-5
CTHR = 1.0 - EPS

_CACHE = {}


def _label_lut():
    """1000 distinct values exactly representable in bf16 (and f32)."""
    vals = list(range(257))  # 0..256 (ints, exact)
    lo = 256
    while len(vals) < 1000:
        step = lo // 128  # within [lo, 2*lo] bf16 resolves multiples of lo/128
        vals.extend(range(lo + step, 2 * lo + 1, step))
        lo *= 2
    return np.array(vals[:1000], dtype=np.float32)


def _build_bass(reps=1):
    import concourse.bacc as bacc
    import concourse.mybir as mybir
    from concourse.tile import TileContext

    f32 = mybir.dt.float32
    bf16 = mybir.dt.bfloat16
    Alu = mybir.AluOpType
    Act = mybir.ActivationFunctionType

    nc = bacc.Bacc("TRN2")
    colT = nc.dram_tensor("colT", [P, KT, B], bf16, kind="ExternalInput")
    rowB = nc.dram_tensor("rowB", [P, NCH, KT, CHW], bf16, kind="ExternalInput")
    tcol = nc.dram_tensor("tcol", [P, NBT], f32, kind="ExternalInput")
    trow = nc.dram_tensor("trow", [1, MS], bf16, kind="ExternalInput")
    cand_o = nc.dram_tensor("cand", [B, NCH, 8], bf16, kind="ExternalOutput")
    sums_o = nc.dram_tensor("sums", [B, 2 * NCH], f32, kind="ExternalOutput")

    with TileContext(nc) as tc:
        with (
            tc.tile_pool(name="const", bufs=1) as const,
            tc.tile_pool(name="rhs", bufs=4) as rhsp,
            tc.tile_pool(name="psum", bufs=2, space="PSUM") as psump,
            tc.tile_pool(name="sim", bufs=3) as simp,
            tc.tile_pool(name="neg", bufs=3) as negp,
            tc.tile_pool(name="scr", bufs=3) as scrp,
        ):
            lhsT = const.tile([P, KT, B], bf16)
            nc.sync.dma_start(lhsT[:], colT[:])
            tcS = const.tile([P, NBT], f32)
            nc.sync.dma_start(tcS[:], tcol[:])
            trR = const.tile([1, MS], bf16)
            nc.sync.dma_start(trR[:], trow[:])
            trB = const.tile([P, MS], bf16)
            # chunked so the broadcasts overlap the first rhs DMAs
            for bc in range(8):
                sl = slice(bc * 1024, (bc + 1) * 1024)
                nc.gpsimd.partition_broadcast(trB[:, sl], trR[:, sl])
            # {0,1} same-label masks, built once: the per-round masking can
            # then be a plain tensor_tensor multiply (DVE 2x mode; the
            # scalar_tensor_tensor form has no fast-mode uops)
            m01 = const.tile([P, NBT, MS], bf16, tag="m01")
            for bt in range(NBT):
                for bc in range(2):
                    sl = slice(bc * (MS // 2), (bc + 1) * (MS // 2))
                    nc.vector.tensor_scalar(
                        out=m01[:, bt, sl],
                        in0=trB[:, sl],
                        scalar1=tcS[:, bt : bt + 1],
                        scalar2=None,
                        op0=Alu.not_equal,
                    )

            sums8 = const.tile([P, NBT, 2 * NCH], f32, tag="sums8")
            candt = const.tile([P, NBT, NCH, 8], bf16, tag="candt")

            for _rep in range(reps):
                for ch in range(NCH):
                    rhs = rhsp.tile([P, KT, CHW], bf16)
                    nc.sync.dma_start(rhs[:], rowB[:, ch])
                    for bt in range(NBT):
                        ps = psump.tile([P, CHW], f32)
                        # kt outer: each lhsT tile stays loaded for 4 matmuls
                        for kt in range(KT):
                            for sub in range(NSUB):
                                nc.tensor.matmul(
                                    ps[:, sub * 512 : (sub + 1) * 512],
                                    lhsT[:, kt, bt * P : (bt + 1) * P],
                                    rhs[:, kt, sub * 512 : (sub + 1) * 512],
                                    start=(kt == 0),
                                    stop=(kt == KT - 1),
                                )
                        # ACT evicts PSUM -> bf16
                        simb = simp.tile([P, CHW], bf16)
                        nc.scalar.activation(simb[:], ps[:], Act.Copy)
                        mbase = ch * CHW
                        # masked copy: negv = m01 * simb (DVE 2x)
                        negc = negp.tile([P, CHW], bf16)
                        nc.vector.tensor_tensor(
                            out=negc[:],
                            in0=m01[:, bt, mbase : mbase + CHW],
                            in1=simb[:],
                            op=Alu.mult,
                        )
                        # S_all accum: sum min(simb, c) (DVE 4x)
                        w = scrp.tile([P, CHW], bf16, tag="w")
                        nc.vector.tensor_scalar(
                            out=w[:],
                            in0=simb[:],
                            scalar1=CTHR,
                            scalar2=None,
                            op0=Alu.min,
                            op1=Alu.add,
                            accum_out=sums8[:, bt, ch : ch + 1],
                        )
                        # S_diff accum: sum min(negv, c) (DVE 4x; Pool
                        # rejects TensorScalarPtr at codegen)
                        v = scrp.tile([P, CHW], bf16, tag="v")
                        nc.vector.tensor_scalar(
                            out=v[:],
                            in0=negc[:],
                            scalar1=CTHR,
                            scalar2=None,
                            op0=Alu.min,
                            op1=Alu.add,
                            accum_out=sums8[:, bt, NCH + ch : NCH + ch + 1],
                        )
                        # pairwise-max fold, then top-8 of the chunk
                        negf = scrp.tile([P, CHW // 2], bf16, tag="negf")
                        nc.vector.tensor_tensor(
                            out=negf[:],
                            in0=negc[:, : CHW // 2],
                            in1=negc[:, CHW // 2 :],
                            op=Alu.max,
                        )
                        nc.vector.max(out=candt[:, bt, ch], in_=negf[:])

            for bt in range(NBT):
                nc.sync.dma_start(
                    sums_o[bt * P : (bt + 1) * P, :], sums8[:, bt]
                )
                nc.sync.dma_start(
                    cand_o[bt * P : (bt + 1) * P], candt[:, bt]
                )

    nc.compile()
    return nc


def _get_bass():
    if "nc" not in _CACHE:
        _CACHE["nc"] = _build_bass()
    return _CACHE["nc"]


def _shard_inputs(inputs_col, targets_col, inputs_row, target_row):
    lut = _label_lut()
    colT = np.ascontiguousarray(
        inputs_col.astype(BF16).T.reshape(KT, P, B).transpose(1, 0, 2)
    )
    tcol = np.ascontiguousarray(
        lut[targets_col.astype(np.int64)].reshape(NBT, P).T
    )
    # [D, M] contiguous transpose (one pass), then pure views per core:
    # rowB[c][p, ch, kt, m] = row[c*MS + ch*CHW + m, kt*128 + p]
    rowT = np.ascontiguousarray(inputs_row.astype(BF16).T)
    rowV = rowT.reshape(KT, P, NCORES, NCH, CHW).transpose(2, 1, 3, 0, 4)
    trF = lut[target_row.astype(np.int64)].astype(BF16).reshape(NCORES, 1, MS)
    return [
        {"colT": colT, "rowB": rowV[c], "tcol": tcol, "trow": trF[c]}
        for c in range(NCORES)
    ]


def _combine(results, targets_col, target_row):
    # [B, 8*NCH*8] candidate pool; [NCORES, B, 2*NCH] partial sums
    cands = np.concatenate(
        [r["cand"].astype(np.float32).reshape(B, NCH * 8) for r in results],
        axis=1,
    )
    sums = np.stack([r["sums"] for r in results])
    counts = np.bincount(target_row.astype(np.int64), minlength=1000)
    n_same = counts[targets_col.astype(np.int64)]
    s_all = sums[:, :, :NCH].sum(axis=(0, 2), dtype=np.float64)
    s_diff = sums[:, :, NCH:].sum(axis=(0, 2), dtype=np.float64)
    pos = n_same.astype(np.float64) * CTHR + (s_diff - s_all)
    neg = np.sort(cands, axis=1)[:, -NEG_TOPK:].sum(axis=1, dtype=np.float64)
    return np.float32(np.mean(pos + neg))


def kernel(inputs_col, targets_col, inputs_row, target_row):
    from concourse.bass_utils import run_bass_kernel_spmd

    nc = _get_bass()
    in_maps = _shard_inputs(inputs_col, targets_col, inputs_row, target_row)
    res = run_bass_kernel_spmd(nc, in_maps, core_ids=list(range(NCORES)))
    return _combine(res.results, targets_col, target_row)
